# revision 6
# baseline (speedup 1.0000x reference)
"""EMA (first-order linear recurrence along T) for x[16, 512, 4096] f32.

v10 "pair-scan, fp8 both ways": ~41.2-43.4 us HW, rel err 6.0e-3 (gate 2e-2).

Sharding: batch B over 8 cores (1024 rows/core, 8 blocks of 128 partitions).
Wire 10.6 MB/core: in = deinterleaved x planes fp8e3; out = ONE fp8-typed
tensor per row [y_even fp8 | y_odd as raw fp16 bytes] (6 KB rows, single
DMA per block; fp16 region written through an AP bitcast view) plus a tiny
fp16 head (first 32 even samples per block, where a^t*x0 exceeds the fp8
budget; odd plane ships full fp16; y0 = x0 patched on host).

- Odds: custom DVE op EMA_PAIRRS_ANT (hand-authored 5-uop chain): Src0 =
  x_odd, Src1 = x_even as separate streams -> one PAIR per cycle at 1x
  mode. Rescale-trick scans (rho *= a^-2, V += rho*w, h *= a^2, y = V*h)
  in the 8-stage pipe; latch-init derives a = 1-s, a^2 into swap flops;
  per-partition f32 carry rides the s0 const slot; two 1-cycle reseed uops
  at each SUB_DIM_DONE (512-pair piece) renormalize V := V*h, h := 1
  in-pipe, so one instruction scans a whole 2048-pair row.
- Evens on the TensorEngine: psum = diag(a) @ y_odd_shifted(fp16 view) +
  diag(s) @ x_even(fp8); the ACT eviction the evens need anyway writes
  PSUM straight to fp8e3 (free conversion). psum pool bufs=4 (all 8 banks)
  so PE runs a full block ahead of the evictions.
- Overlap: full input prefetch (bufs=n_blocks), one 4 KB-row input DMA per
  block, carries in one DVE copy up front, tail block split (scan halves +
  fine evictions + odd-half output first).
- Perf anatomy: 7.2 preamble + ~1.4 trigger->data + ~29 us DMA-saturated
  payload (10.6 MB at 16 queues) + ~2.5 postamble. DVE 21 us / PE 19 /
  ACT 21 hide under the DMA. Rejected: full-fp8 output (odd-plane
  conversion costs more engine time than it saves), GPSIMD offloads
  (8.8 us/block software CAST, no PSUM access), packed 2x DVE modes.
"""

import numpy as np
import ml_dtypes

import concourse.bacc as bacc
import concourse.mybir as mybir
import concourse.tile as tile
from concourse.bass_utils import run_bass_kernel_spmd

B, C, T = 16, 512, 4096
N_CORES = 8
B_PER = B // N_CORES          # 2 batches per core
ROWS = B_PER * C              # 1024 rows per core
P = 128
N_BLOCKS = ROWS // P          # 8 row blocks
K = T // 2                    # 2048 pairs per row
KP = K // 2                   # 1024 pairs per piece (rescale range limit)
KP2 = KP // 2                 # quarter piece (block-0 fast start)
KQ = 512                      # pairs per in-pipe reseed piece
HEAD = 32                     # fp16 head columns per output plane
MM = 512                      # matmul moving chunk (1 PSUM bank of f32)

F32 = mybir.dt.float32
F16 = mybir.dt.float16
F8 = mybir.dt.float8e3
ACT_COPY = mybir.ActivationFunctionType.Copy
OP = mybir.AluOpType

_OPS = {}


def _register_ema_pair_op(name="EMA_PAIRRS_ANT"):
    """Self-reseeding pair-scan EMA op; see dev_test_reseed.py for layout."""
    from concourse.dve_spec import Spec, Src0, Src1, C0, C1, C2
    from concourse.dve_ops import (
        DveOp, OPS, CUSTOM_DVE_SPECS, _SUB_OPCODE_FOR_NAME,
        _CUSTOM_DVE_ROW_BASE, get_dve_sub_opcode, _COMPILE_CACHE,
    )
    from concourse.dve_uop import (
        DveOpSpec, UopConfig, AluOp, AluInp, InpSel,
        OutPath, OutSel, Trigger, DelayInp, ENABLE,
    )
    if name in _OPS:
        return _OPS[name]

    PREV = AluInp.PREV_ALU_OUT
    CURR = AluInp.CURR_ALU_OUT
    SWAP = AluInp.CURR_SWAP_OUT
    D = lambda n: AluInp(int(AluInp.PREV_DELAY_0) + n)

    # lanes: d0=xo d1=xe d2=a^-2(C2) d3=w then m d4=h
    latch = UopConfig()
    latch.enable_input(InpSel.ONE_F32, 1)
    latch.enable_input(InpSel.CONST_1, 2)
    ldp = latch.datapath_config
    for st in range(8):
        ldp[st].pass_through_delay(0, 1)
    ldp[0].enable_alu(AluOp.SUBTRACT, D(0), D(1))
    ldp[1].enable_alu(AluOp.BYPASS, PREV, PREV)
    ldp[1].swap_enable = ENABLE
    ldp[2].enable_alu(AluOp.MULTIPLY, PREV, PREV)
    for st in (3, 4):
        ldp[st].enable_alu(AluOp.BYPASS, PREV, PREV)
    ldp[5].enable_alu(AluOp.BYPASS, PREV, PREV)
    ldp[5].swap_enable = ENABLE
    latch.repeat_count = 1
    latch.trigger = (Trigger.COUNT, Trigger.NONE, Trigger.NONE)
    latch.next_uop = (1, 0, 0)

    seed = UopConfig()
    seed.enable_input(InpSel.CONST_1, 1)   # d0 = s
    seed.enable_input(InpSel.CONST_0, 2)   # d1 = carry
    seed.enable_input(InpSel.ONE_F32, 3)   # d2 = 1
    sdp = seed.datapath_config
    for st in range(8):
        sdp[st].pass_through_delay(0, 1, 2)
    sdp[3].enable_alu(AluOp.BYPASS, D(0), D(0))   # rho := s
    sdp[5].enable_alu(AluOp.BYPASS, D(2), D(2))   # h := 1
    sdp[6].enable_alu(AluOp.BYPASS, D(1), D(1))   # V := carry
    seed.repeat_count = 1
    seed.trigger = (Trigger.COUNT, Trigger.NONE, Trigger.NONE)
    seed.next_uop = (2, 0, 0)

    steady = UopConfig()
    steady.enable_input(InpSel.SRC_0, 1)
    steady.enable_input(InpSel.SRC_1, 2)
    steady.enable_input(InpSel.CONST_2, 3)
    dp = steady.datapath_config
    for st in range(8):
        dp[st].pass_through_delay(0, 1, 2, 3, 4)
    dp[1].enable_alu(AluOp.MULTIPLY, D(1), SWAP)          # t = a*xe
    dp[2].enable_alu(AluOp.ADD, PREV, D(0))               # w = t+xo
    dp[3].enable_alu(AluOp.MULTIPLY, CURR, D(2))          # rho *= a^-2
    dp[3].enable_delay_from_src(DelayInp.PREV_ALU_OUT, 3)  # d3 := w
    dp[4].enable_alu(AluOp.MULTIPLY, PREV, D(3))          # m = rho*w
    dp[5].enable_alu(AluOp.MULTIPLY, CURR, SWAP)          # h *= a^2
    dp[5].enable_delay_from_src(DelayInp.PREV_ALU_OUT, 3)  # d3 := m
    dp[6].enable_alu(AluOp.ADD, CURR, D(3))               # V += m
    dp[6].enable_delay_from_src(DelayInp.PREV_ALU_OUT, 4)  # d4 := h
    dp[7].enable_alu(AluOp.MULTIPLY, PREV, D(4))          # y = V*h
    steady.enable_output(OutSel.ALU_OUT, OutPath.WR0_LO)
    steady.require_inp0 = ENABLE
    steady.require_inp1 = ENABLE
    steady.trigger = (Trigger.SRC_TENSOR_DONE, Trigger.SUB_DIM_DONE,
                      Trigger.NONE)
    steady.next_uop = (0, 3, 0)

    def mk_rs(st5_op, st6_op, last):
        rs = UopConfig()
        rs.enable_input(InpSel.CONST_1, 1)   # d0 = s
        rs.enable_input(InpSel.ONE_F32, 3)   # d2 = 1
        rdp = rs.datapath_config
        for st in range(8):
            rdp[st].pass_through_delay(0, 1, 2)
        rdp[3].enable_alu(AluOp.BYPASS, D(0), D(0))       # rho := s
        st5_op(rdp[5])
        st6_op(rdp[6])
        rs.repeat_count = 1
        rs.trigger = (Trigger.COUNT, Trigger.NONE, Trigger.NONE)
        rs.next_uop = (4 if not last else 2, 0, 0)
        return rs

    rs1 = mk_rs(
        lambda b: b.enable_alu(AluOp.BYPASS, CURR, CURR),       # h pass
        lambda b: b.enable_alu(AluOp.MULTIPLY, CURR, PREV),     # V *= h_bnd
        last=False)
    rs2 = mk_rs(
        lambda b: b.enable_alu(AluOp.BYPASS, D(2), D(2)),       # h := 1
        lambda b: b.enable_alu(AluOp.BYPASS, CURR, CURR),       # V hold
        last=True)

    def ref(in0, in1, s0, s1, imm2):
        s = np.float64(s1)
        a = 1.0 - s
        w = a * in1.astype(np.float64) + in0.astype(np.float64)
        w = w.reshape(w.shape[0], -1)
        kk = np.arange(w.shape[-1]) + 1.0
        rho = s * (1.0 / a) ** (2 * kk)
        V = np.asarray(s0).reshape(-1, 1) + np.cumsum(rho * w, axis=-1)
        return (V * a ** (2 * kk)).astype(np.float32)

    spec = Spec(body=(Src0 + Src1 * C0) * C1 * C2, reference=ref)
    op = DveOp(name, spec, subdim=True, uops_sha={})
    OPS.append(op)
    CUSTOM_DVE_SPECS[name] = spec
    _SUB_OPCODE_FOR_NAME[name] = _CUSTOM_DVE_ROW_BASE + len(OPS) - 1
    _COMPILE_CACHE[(name, "v3")] = DveOpSpec(
        name=name, opcode=get_dve_sub_opcode(name),
        uops=[latch, seed, steady, rs1, rs2], rd1_en=True)
    _OPS[name] = op
    return op


def build_fast_v2(s0v, b_per=B_PER, c=C):
    """Pair-scan kernel: fp8 in, fp16 odd plane / fp8 even plane out.

    Odds: self-reseeding DVE pair-scan writes fp16 (also the recon input),
    shipped as-is. Evens: PE diag-matmul recon; the ACT eviction the evens
    need anyway writes PSUM straight to fp8e3 (free conversion), plus a
    small fp16 head (first HEAD even samples per block, where a^t*x0 can
    exceed the fp8 error budget). Wire: 4.2 in + 4.2 + 2.1 + 0.13 out
    = 10.6 MB/core.
    """
    ema = _register_ema_pair_op()
    a0 = 1.0 - s0v
    rows = b_per * c
    n_blocks = rows // P
    imm_a2i = float((1.0 / np.float64(a0)) ** 2)

    nc = bacc.Bacc("TRN2", target_bir_lowering=False, debug=False)

    x_in = nc.dram_tensor("xp", [rows, 2 * K], F8, kind="ExternalInput")
    cst_in = nc.dram_tensor("cst", [P, 2 * P + n_blocks], F16,
                            kind="ExternalInput")
    yb_out = nc.dram_tensor("yb", [rows, 3 * K], F8, kind="ExternalOutput")
    h_out = nc.dram_tensor("yh", [P, HEAD * n_blocks], F16,
                           kind="ExternalOutput")

    xr = x_in.ap()
    ybr = yb_out.ap()

    with tile.TileContext(nc) as tc:
        with (
            tc.tile_pool(name="const", bufs=1) as cpool,
            tc.tile_pool(name="xpool", bufs=n_blocks) as xpool,
            tc.tile_pool(name="ypool", bufs=n_blocks) as ypool,
            tc.tile_pool(name="y8pool", bufs=n_blocks) as y8pool,
            tc.psum_pool(name="ps", bufs=4) as psp,
        ):
            cst = cpool.tile([P, 2 * P + n_blocks], F16)  # [da | ds | x0]
            cr = cpool.tile([P, n_blocks + 1], F32)
            yh = cpool.tile([P, HEAD * n_blocks], F16)
            nc.sync.dma_start(cst[:], cst_in.ap())
            da = cst[:, 0:P]
            ds = cst[:, P:2 * P]
            x0t = cst[:, 2 * P:2 * P + n_blocks]
            # all piece-1 carries in one DVE copy (DVE is idle pre-scan;
            # keeps ACT free for evictions)
            nc.vector.tensor_copy(cr[:, 0:n_blocks], x0t[:])

            outs = []

            def recon_evens(k, xt, yo, y8, fine):
                """psum = diag(a) @ y_shifted + diag(s) @ x_even -> fp8;
                first chunk also drops an fp16 head of the even samples."""
                step = MM if fine else 2 * MM
                for j in range(0, K, step):
                    ps = psp.tile([P, step], F32)
                    for jj in range(j, j + step, MM):
                        o = jj - j
                        if jj == 0:
                            nc.tensor.matmul(
                                ps[:, 1:MM], da, yo[:, 0:MM - 1],
                                start=True, stop=False)
                        else:
                            nc.tensor.matmul(
                                ps[:, o:o + MM], da,
                                yo[:, jj - 1:jj + MM - 1],
                                start=True, stop=False)
                        nc.tensor.matmul(
                            ps[:, o:o + MM], ds, xt[:, K + jj:K + jj + MM],
                            start=False, stop=True)
                    if j == 0:
                        nc.scalar.activation(
                            yh[:, HEAD * k:HEAD * (k + 1)], ps[:, 0:HEAD],
                            ACT_COPY)
                    nc.scalar.activation(y8[:, j:j + step], ps[:], ACT_COPY)

            for k in range(n_blocks):
                r0 = k * P
                xt = xpool.tile([P, 2 * K], F8)
                nc.sync.dma_start(xt[:], xr[r0:r0 + P, :])

                yb = ypool.tile([P, 3 * K], F8)  # [ye fp8 | yo fp16-bytes]
                yt = yb[:, K:3 * K].bitcast(F16)  # y_odd fp16 view [P, K]
                y8 = yb[:, 0:K]                   # y_even fp8
                ca = cr[:, k:k + 1]
                if k == n_blocks - 1:
                    cq = cr[:, n_blocks:n_blocks + 1]
                    nc.vector._custom_dve(
                        ema, out=yt[:, 0:KP],
                        in0=xt[:, 0:KP].rearrange("p (g n) -> p g n", n=KQ),
                        in1=xt[:, K:K + KP], s0=ca, s1=float(s0v),
                        imm2=imm_a2i)
                    nc.vector.tensor_copy(cq, yt[:, KP - 1:KP])
                    nc.vector._custom_dve(
                        ema, out=yt[:, KP:K],
                        in0=xt[:, KP:K].rearrange("p (g n) -> p g n", n=KQ),
                        in1=xt[:, K + KP:2 * K], s0=cq, s1=float(s0v),
                        imm2=imm_a2i)
                else:
                    nc.vector._custom_dve(
                        ema, out=yt[:, 0:K],
                        in0=xt[:, 0:K].rearrange("p (g n) -> p g n", n=KQ),
                        in1=xt[:, K:2 * K], s0=ca, s1=float(s0v),
                        imm2=imm_a2i)
                recon_evens(k, xt, yt, y8, fine=(k == n_blocks - 1))
                if k == n_blocks - 1:
                    # odd half is ready at scan end, ~4 us before the evens
                    outs.append((ybr[r0:r0 + P, K:3 * K], yb[:, K:3 * K]))
                    outs.append((ybr[r0:r0 + P, 0:K], yb[:, 0:K]))
                else:
                    outs.append((ybr[r0:r0 + P, :], yb[:]))
            for dst, src in outs:
                nc.sync.dma_start(dst, src)
            nc.sync.dma_start(h_out.ap(), yh[:])
    nc.compile()
    return nc


def build_fallback(b_per=B_PER, c=C, t=T):
    """Stock-scan kernel (v2): correct for any weights."""
    rows = b_per * c
    n_blocks = rows // P
    c_blocks = c // P
    th = t // 2

    nc = bacc.Bacc("TRN2", target_bir_lowering=False, debug=False)

    x_in = nc.dram_tensor("x", [b_per, c, t], F16, kind="ExternalInput")
    s_in = nc.dram_tensor("s32", [c], F32, kind="ExternalInput")
    a_in = nc.dram_tensor("a16", [c], F16, kind="ExternalInput")
    y_out = nc.dram_tensor("out", [b_per, c, t], F16, kind="ExternalOutput")

    xr = x_in.ap().rearrange("b c t -> (b c) t")
    yr = y_out.ap().rearrange("b c t -> (b c) t")
    sr = s_in.ap().rearrange("(j p) -> p j", p=P)
    ar = a_in.ap().rearrange("(j p) -> p j", p=P)

    with tile.TileContext(nc) as tc:
        with (
            tc.tile_pool(name="const", bufs=1) as cpool,
            tc.tile_pool(name="xp", bufs=6) as xpool,
            tc.tile_pool(name="xh", bufs=4) as hpool,
        ):
            s4 = cpool.tile([P, c_blocks], F32)
            a4 = cpool.tile([P, c_blocks], F16)
            nc.sync.dma_start(s4[:], sr)
            nc.sync.dma_start(a4[:], ar)

            def premul_scan(xt, lo, hi, j, first, init):
                a, b = lo + (1 if first else 0), hi
                nc.scalar.activation(
                    xt[:, a:b], xt[:, a:b], ACT_COPY, scale=s4[:, j:j + 1])
                nc.vector.tensor_tensor_scan(
                    xt[:, lo:hi],
                    a4[:, j:j + 1].to_broadcast((P, hi - lo)),
                    xt[:, lo:hi],
                    init,
                    OP.mult,
                    OP.add,
                )

            split_blocks = (0, n_blocks - 1)
            outs = []
            for k in range(n_blocks):
                j = k % c_blocks
                r0 = k * P
                if k in split_blocks:
                    xa = hpool.tile([P, th], F16)
                    xb = hpool.tile([P, th], F16)
                    nc.sync.dma_start(xa[:], xr[r0:r0 + P, 0:th])
                    nc.sync.dma_start(xb[:], xr[r0:r0 + P, th:t])
                    premul_scan(xa, 0, th, j, True, 0.0)
                    outs.append((yr[r0:r0 + P, 0:th], xa[:]))
                    premul_scan(xb, 0, th, j, False, xa[:, th - 1:th])
                    outs.append((yr[r0:r0 + P, th:t], xb[:]))
                else:
                    xt = xpool.tile([P, t], F16)
                    nc.sync.dma_start(xt[:], xr[r0:r0 + P, :])
                    premul_scan(xt, 0, t, j, True, 0.0)
                    outs.append((yr[r0:r0 + P, :], xt[:]))
            for dst, src in outs:
                nc.sync.dma_start(dst, src)
    nc.compile()
    return nc


_NC_CACHE = {}


def _enable_jax_compile_cache():
    try:
        import jax
        jax.config.update("jax_compilation_cache_dir", "/tmp/jax_neff_cache")
        jax.config.update("jax_persistent_cache_min_compile_time_secs", 1.0)
    except Exception:
        pass


def _get_nc(kind, *args):
    key = (kind,) + tuple(args)
    if key not in _NC_CACHE:
        _enable_jax_compile_cache()
        _NC_CACHE[key] = (build_fast_v2(*args) if kind == "fast"
                          else build_fallback())
    return _NC_CACHE[key]


def _fast_path_ok(s):
    if not np.all(s == s[0]):
        return False
    s0 = float(s[0])
    a0 = 1.0 - s0
    if not (0.0 < s0 < 1.0) or a0 <= 0.0:
        return False
    try:
        lo = a0 ** (T // 2)
        hi = (a0 ** -(T // 2)) * s0 * 125
    except OverflowError:
        return False
    return lo > 1e-37 and hi < 3e37


def kernel(x, weights, _run_kwargs=None):
    x32 = np.asarray(x, dtype=np.float32)
    s = np.clip(np.asarray(weights, dtype=np.float64), 0.0, 1.0)

    if _fast_path_ok(s) and np.abs(x32).max() < 15.0:
        s0 = float(s[0])
        a0 = 1.0 - s0
        xr = x32.reshape(B * C, T)
        x8 = xr.astype(ml_dtypes.float8_e3m4).reshape(B * C, K, 2)
        xe8, xo8 = x8[..., 0], x8[..., 1]
        packed = np.ascontiguousarray(np.concatenate([xo8, xe8], axis=1))
        x0 = xr[:, 0].astype(np.float16).reshape(N_CORES, N_BLOCKS, P) \
            .transpose(0, 2, 1)                       # [core][P, n_blocks]
        da = (np.eye(P) * a0).astype(np.float16)
        ds = (np.eye(P) * s0).astype(np.float16)
        cst = [np.ascontiguousarray(np.concatenate([da, ds, x0[i]], axis=1))
               for i in range(N_CORES)]
        nc = _get_nc("fast", s0)
        in_maps = [
            {"xp": packed[i * ROWS:(i + 1) * ROWS], "cst": cst[i]}
            for i in range(N_CORES)
        ]
        res = run_bass_kernel_spmd(
            nc, in_maps, core_ids=list(range(N_CORES)), **(_run_kwargs or {})
        )
        yb = np.concatenate(
            [np.ascontiguousarray(res.results[i]["yb"])
             for i in range(N_CORES)], axis=0)
        ye = yb[:, 0:K].astype(np.float32)
        yo = yb[:, K:3 * K].view(np.float16).astype(np.float32)
        out = np.empty((B * C, T), dtype=np.float32)
        out[:, 0::2] = ye
        out[:, 1::2] = yo
        # fp16 heads: first HEAD even samples per block
        for i in range(N_CORES):
            yh = res.results[i]["yh"].astype(np.float32)  # [P, HEAD*NB]
            for kb in range(N_BLOCKS):
                rws = slice(i * ROWS + kb * P, i * ROWS + (kb + 1) * P)
                out[rws, 0:2 * HEAD:2] = yh[:, HEAD * kb:HEAD * (kb + 1)]
        out[:, 0] = xr[:, 0]          # y_0 = x_0 exactly
        out = out.reshape(B, C, T)
    else:
        x16 = np.ascontiguousarray(x32.astype(np.float16))
        nc = _get_nc("fallback")
        s32 = s.astype(np.float32)
        a16 = (1.0 - s32).astype(np.float16)
        in_maps = [
            {"x": x16[i * B_PER:(i + 1) * B_PER], "s32": s32, "a16": a16}
            for i in range(N_CORES)
        ]
        res = run_bass_kernel_spmd(
            nc, in_maps, core_ids=list(range(N_CORES)), **(_run_kwargs or {})
        )
        out16 = np.concatenate(
            [res.results[i]["out"] for i in range(N_CORES)], axis=0)
        out = out16.astype(np.float32)
    if _run_kwargs:
        kernel.last_results = res
    return out


# revision 7
# speedup vs baseline: 1.0116x; 1.0116x over previous
"""EMA (first-order linear recurrence along T) for x[16, 512, 4096] f32.

v10 "pair-scan, fp8 both ways": ~41.2-43.4 us HW, rel err 6.0e-3 (gate 2e-2).

Sharding: batch B over 8 cores (1024 rows/core, 8 blocks of 128 partitions).
Wire 10.6 MB/core: in = deinterleaved x planes fp8e3; out = ONE fp8-typed
tensor per row [y_even fp8 | y_odd as raw fp16 bytes] (6 KB rows, single
DMA per block; fp16 region written through an AP bitcast view) plus a tiny
fp16 head (first 32 even samples per block, where a^t*x0 exceeds the fp8
budget; odd plane ships full fp16; y0 = x0 patched on host).

- Odds: custom DVE op EMA_PAIRRS_ANT (hand-authored 5-uop chain): Src0 =
  x_odd, Src1 = x_even as separate streams -> one PAIR per cycle at 1x
  mode. Rescale-trick scans (rho *= a^-2, V += rho*w, h *= a^2, y = V*h)
  in the 8-stage pipe; latch-init derives a = 1-s, a^2 into swap flops;
  per-partition f32 carry rides the s0 const slot; two 1-cycle reseed uops
  at each SUB_DIM_DONE (512-pair piece) renormalize V := V*h, h := 1
  in-pipe, so one instruction scans a whole 2048-pair row.
- Evens on the TensorEngine: psum = diag(a) @ y_odd_shifted(fp16 view) +
  diag(s) @ x_even(fp8); the ACT eviction the evens need anyway writes
  PSUM straight to fp8e3 (free conversion). psum pool bufs=4 (all 8 banks)
  so PE runs a full block ahead of the evictions.
- Overlap: full input prefetch (bufs=n_blocks), one 4 KB-row input DMA per
  block, carries in one DVE copy up front, tail block: scan halves, odd half ships at scan end, and its evens
  come from a short 3-stage DVE recon op (the DVE is idle post-scan),
  skipping the PE->PSUM->ACT chain entirely.
- Perf anatomy: 7.2 preamble + ~1.4 trigger->data + ~29 us DMA-saturated
  payload (10.6 MB at 16 queues) + ~2.5 postamble. DVE 21 us / PE 19 /
  ACT 21 hide under the DMA. Rejected: full-fp8 output (odd-plane
  conversion costs more engine time than it saves), GPSIMD offloads
  (8.8 us/block software CAST, no PSUM access), packed 2x DVE modes.
"""

import numpy as np
import ml_dtypes

import concourse.bacc as bacc
import concourse.mybir as mybir
import concourse.tile as tile
from concourse.bass_utils import run_bass_kernel_spmd

B, C, T = 16, 512, 4096
N_CORES = 8
B_PER = B // N_CORES          # 2 batches per core
ROWS = B_PER * C              # 1024 rows per core
P = 128
N_BLOCKS = ROWS // P          # 8 row blocks
K = T // 2                    # 2048 pairs per row
KP = K // 2                   # 1024 pairs per piece (rescale range limit)
KP2 = KP // 2                 # quarter piece (block-0 fast start)
KQ = 512                      # pairs per in-pipe reseed piece
HEAD = 32                     # fp16 head columns per output plane
MM = 512                      # matmul moving chunk (1 PSUM bank of f32)

F32 = mybir.dt.float32
F16 = mybir.dt.float16
F8 = mybir.dt.float8e3
ACT_COPY = mybir.ActivationFunctionType.Copy
OP = mybir.AluOpType

_OPS = {}


def _register_ema_pair_op(name="EMA_PAIRRS_ANT"):
    """Self-reseeding pair-scan EMA op; see dev_test_reseed.py for layout."""
    from concourse.dve_spec import Spec, Src0, Src1, C0, C1, C2
    from concourse.dve_ops import (
        DveOp, OPS, CUSTOM_DVE_SPECS, _SUB_OPCODE_FOR_NAME,
        _CUSTOM_DVE_ROW_BASE, get_dve_sub_opcode, _COMPILE_CACHE,
    )
    from concourse.dve_uop import (
        DveOpSpec, UopConfig, AluOp, AluInp, InpSel,
        OutPath, OutSel, Trigger, DelayInp, ENABLE,
    )
    if name in _OPS:
        return _OPS[name]

    PREV = AluInp.PREV_ALU_OUT
    CURR = AluInp.CURR_ALU_OUT
    SWAP = AluInp.CURR_SWAP_OUT
    D = lambda n: AluInp(int(AluInp.PREV_DELAY_0) + n)

    # lanes: d0=xo d1=xe d2=a^-2(C2) d3=w then m d4=h
    latch = UopConfig()
    latch.enable_input(InpSel.ONE_F32, 1)
    latch.enable_input(InpSel.CONST_1, 2)
    ldp = latch.datapath_config
    for st in range(8):
        ldp[st].pass_through_delay(0, 1)
    ldp[0].enable_alu(AluOp.SUBTRACT, D(0), D(1))
    ldp[1].enable_alu(AluOp.BYPASS, PREV, PREV)
    ldp[1].swap_enable = ENABLE
    ldp[2].enable_alu(AluOp.MULTIPLY, PREV, PREV)
    for st in (3, 4):
        ldp[st].enable_alu(AluOp.BYPASS, PREV, PREV)
    ldp[5].enable_alu(AluOp.BYPASS, PREV, PREV)
    ldp[5].swap_enable = ENABLE
    latch.repeat_count = 1
    latch.trigger = (Trigger.COUNT, Trigger.NONE, Trigger.NONE)
    latch.next_uop = (1, 0, 0)

    seed = UopConfig()
    seed.enable_input(InpSel.CONST_1, 1)   # d0 = s
    seed.enable_input(InpSel.CONST_0, 2)   # d1 = carry
    seed.enable_input(InpSel.ONE_F32, 3)   # d2 = 1
    sdp = seed.datapath_config
    for st in range(8):
        sdp[st].pass_through_delay(0, 1, 2)
    sdp[3].enable_alu(AluOp.BYPASS, D(0), D(0))   # rho := s
    sdp[5].enable_alu(AluOp.BYPASS, D(2), D(2))   # h := 1
    sdp[6].enable_alu(AluOp.BYPASS, D(1), D(1))   # V := carry
    seed.repeat_count = 1
    seed.trigger = (Trigger.COUNT, Trigger.NONE, Trigger.NONE)
    seed.next_uop = (2, 0, 0)

    steady = UopConfig()
    steady.enable_input(InpSel.SRC_0, 1)
    steady.enable_input(InpSel.SRC_1, 2)
    steady.enable_input(InpSel.CONST_2, 3)
    dp = steady.datapath_config
    for st in range(8):
        dp[st].pass_through_delay(0, 1, 2, 3, 4)
    dp[1].enable_alu(AluOp.MULTIPLY, D(1), SWAP)          # t = a*xe
    dp[2].enable_alu(AluOp.ADD, PREV, D(0))               # w = t+xo
    dp[3].enable_alu(AluOp.MULTIPLY, CURR, D(2))          # rho *= a^-2
    dp[3].enable_delay_from_src(DelayInp.PREV_ALU_OUT, 3)  # d3 := w
    dp[4].enable_alu(AluOp.MULTIPLY, PREV, D(3))          # m = rho*w
    dp[5].enable_alu(AluOp.MULTIPLY, CURR, SWAP)          # h *= a^2
    dp[5].enable_delay_from_src(DelayInp.PREV_ALU_OUT, 3)  # d3 := m
    dp[6].enable_alu(AluOp.ADD, CURR, D(3))               # V += m
    dp[6].enable_delay_from_src(DelayInp.PREV_ALU_OUT, 4)  # d4 := h
    dp[7].enable_alu(AluOp.MULTIPLY, PREV, D(4))          # y = V*h
    steady.enable_output(OutSel.ALU_OUT, OutPath.WR0_LO)
    steady.require_inp0 = ENABLE
    steady.require_inp1 = ENABLE
    steady.trigger = (Trigger.SRC_TENSOR_DONE, Trigger.SUB_DIM_DONE,
                      Trigger.NONE)
    steady.next_uop = (0, 3, 0)

    def mk_rs(st5_op, st6_op, last):
        rs = UopConfig()
        rs.enable_input(InpSel.CONST_1, 1)   # d0 = s
        rs.enable_input(InpSel.ONE_F32, 3)   # d2 = 1
        rdp = rs.datapath_config
        for st in range(8):
            rdp[st].pass_through_delay(0, 1, 2)
        rdp[3].enable_alu(AluOp.BYPASS, D(0), D(0))       # rho := s
        st5_op(rdp[5])
        st6_op(rdp[6])
        rs.repeat_count = 1
        rs.trigger = (Trigger.COUNT, Trigger.NONE, Trigger.NONE)
        rs.next_uop = (4 if not last else 2, 0, 0)
        return rs

    rs1 = mk_rs(
        lambda b: b.enable_alu(AluOp.BYPASS, CURR, CURR),       # h pass
        lambda b: b.enable_alu(AluOp.MULTIPLY, CURR, PREV),     # V *= h_bnd
        last=False)
    rs2 = mk_rs(
        lambda b: b.enable_alu(AluOp.BYPASS, D(2), D(2)),       # h := 1
        lambda b: b.enable_alu(AluOp.BYPASS, CURR, CURR),       # V hold
        last=True)

    def ref(in0, in1, s0, s1, imm2):
        s = np.float64(s1)
        a = 1.0 - s
        w = a * in1.astype(np.float64) + in0.astype(np.float64)
        w = w.reshape(w.shape[0], -1)
        kk = np.arange(w.shape[-1]) + 1.0
        rho = s * (1.0 / a) ** (2 * kk)
        V = np.asarray(s0).reshape(-1, 1) + np.cumsum(rho * w, axis=-1)
        return (V * a ** (2 * kk)).astype(np.float32)

    spec = Spec(body=(Src0 + Src1 * C0) * C1 * C2, reference=ref)
    op = DveOp(name, spec, subdim=True, uops_sha={})
    OPS.append(op)
    CUSTOM_DVE_SPECS[name] = spec
    _SUB_OPCODE_FOR_NAME[name] = _CUSTOM_DVE_ROW_BASE + len(OPS) - 1
    _COMPILE_CACHE[(name, "v3")] = DveOpSpec(
        name=name, opcode=get_dve_sub_opcode(name),
        uops=[latch, seed, steady, rs1, rs2], rd1_en=True)
    _OPS[name] = op
    return op


def _register_recon_op(name="EMA_RECON_ANT"):
    """y_even = a*y_c + s*x_e as a 3-stage DVE op (stock lower()); used for
    the tail block so its evens skip the PE->PSUM->ACT chain after the
    last scan (DVE is idle then)."""
    from concourse.dve_spec import Spec, Src0, Src1, C0, C1, lower
    from concourse.dve_ops import (
        DveOp, OPS, CUSTOM_DVE_SPECS, _SUB_OPCODE_FOR_NAME,
        _CUSTOM_DVE_ROW_BASE, get_dve_sub_opcode, _COMPILE_CACHE,
    )
    from concourse.dve_uop import DveOpSpec
    if name in _OPS:
        return _OPS[name]
    spec = Spec(
        body=Src0 * C0 + Src1 * C1,
        reference=lambda in0, in1, s0, s1, imm2: (
            in0.astype(np.float32) * s0 + in1.astype(np.float32) * s1),
    )
    op = DveOp(name, spec, subdim=False, uops_sha={})
    OPS.append(op)
    CUSTOM_DVE_SPECS[name] = spec
    _SUB_OPCODE_FOR_NAME[name] = _CUSTOM_DVE_ROW_BASE + len(OPS) - 1
    _COMPILE_CACHE[(name, "v3")] = DveOpSpec(
        name=name, opcode=get_dve_sub_opcode(name),
        uops=lower(spec, ver="v3"), rd1_en=True)
    _OPS[name] = op
    return op


def build_fast_v2(s0v, b_per=B_PER, c=C):
    """Pair-scan kernel: fp8 in, fp16 odd plane / fp8 even plane out.

    Odds: self-reseeding DVE pair-scan writes fp16 (also the recon input),
    shipped as-is. Evens: PE diag-matmul recon; the ACT eviction the evens
    need anyway writes PSUM straight to fp8e3 (free conversion), plus a
    small fp16 head (first HEAD even samples per block, where a^t*x0 can
    exceed the fp8 error budget). Wire: 4.2 in + 4.2 + 2.1 + 0.13 out
    = 10.6 MB/core.
    """
    ema = _register_ema_pair_op()
    rec = _register_recon_op()
    a0 = 1.0 - s0v
    rows = b_per * c
    n_blocks = rows // P
    imm_a2i = float((1.0 / np.float64(a0)) ** 2)

    nc = bacc.Bacc("TRN2", target_bir_lowering=False, debug=False)

    x_in = nc.dram_tensor("xp", [rows, 2 * K], F8, kind="ExternalInput")
    cst_in = nc.dram_tensor("cst", [P, 2 * P + n_blocks], F16,
                            kind="ExternalInput")
    yb_out = nc.dram_tensor("yb", [rows, 3 * K], F8, kind="ExternalOutput")
    h_out = nc.dram_tensor("yh", [P, HEAD * n_blocks], F16,
                           kind="ExternalOutput")

    xr = x_in.ap()
    ybr = yb_out.ap()

    with tile.TileContext(nc) as tc:
        with (
            tc.tile_pool(name="const", bufs=1) as cpool,
            tc.tile_pool(name="xpool", bufs=n_blocks) as xpool,
            tc.tile_pool(name="ypool", bufs=n_blocks) as ypool,
            tc.tile_pool(name="y8pool", bufs=n_blocks) as y8pool,
            tc.psum_pool(name="ps", bufs=4) as psp,
        ):
            cst = cpool.tile([P, 2 * P + n_blocks], F16)  # [da | ds | x0]
            cr = cpool.tile([P, n_blocks + 1], F32)
            yh = cpool.tile([P, HEAD * n_blocks], F16)
            nc.sync.dma_start(cst[:], cst_in.ap())
            da = cst[:, 0:P]
            ds = cst[:, P:2 * P]
            x0t = cst[:, 2 * P:2 * P + n_blocks]
            # all piece-1 carries in one DVE copy (DVE is idle pre-scan;
            # keeps ACT free for evictions)
            nc.vector.tensor_copy(cr[:, 0:n_blocks], x0t[:])

            outs = []

            def recon_evens(k, xt, yo, y8, fine):
                """psum = diag(a) @ y_shifted + diag(s) @ x_even -> fp8;
                first chunk also drops an fp16 head of the even samples."""
                step = MM if fine else 2 * MM
                for j in range(0, K, step):
                    ps = psp.tile([P, step], F32)
                    for jj in range(j, j + step, MM):
                        o = jj - j
                        if jj == 0:
                            nc.tensor.matmul(
                                ps[:, 1:MM], da, yo[:, 0:MM - 1],
                                start=True, stop=False)
                        else:
                            nc.tensor.matmul(
                                ps[:, o:o + MM], da,
                                yo[:, jj - 1:jj + MM - 1],
                                start=True, stop=False)
                        nc.tensor.matmul(
                            ps[:, o:o + MM], ds, xt[:, K + jj:K + jj + MM],
                            start=False, stop=True)
                    if j == 0:
                        nc.scalar.activation(
                            yh[:, HEAD * k:HEAD * (k + 1)], ps[:, 0:HEAD],
                            ACT_COPY)
                    nc.scalar.activation(y8[:, j:j + step], ps[:], ACT_COPY)

            for k in range(n_blocks):
                r0 = k * P
                xt = xpool.tile([P, 2 * K], F8)
                nc.sync.dma_start(xt[:], xr[r0:r0 + P, :])

                yb = ypool.tile([P, 3 * K], F8)  # [ye fp8 | yo fp16-bytes]
                yt = yb[:, K:3 * K].bitcast(F16)  # y_odd fp16 view [P, K]
                y8 = yb[:, 0:K]                   # y_even fp8
                ca = cr[:, k:k + 1]
                if k == n_blocks - 1:
                    cq = cr[:, n_blocks:n_blocks + 1]
                    nc.vector._custom_dve(
                        ema, out=yt[:, 0:KP],
                        in0=xt[:, 0:KP].rearrange("p (g n) -> p g n", n=KQ),
                        in1=xt[:, K:K + KP], s0=ca, s1=float(s0v),
                        imm2=imm_a2i)
                    nc.vector.tensor_copy(cq, yt[:, KP - 1:KP])
                    nc.vector._custom_dve(
                        ema, out=yt[:, KP:K],
                        in0=xt[:, KP:K].rearrange("p (g n) -> p g n", n=KQ),
                        in1=xt[:, K + KP:2 * K], s0=cq, s1=float(s0v),
                        imm2=imm_a2i)
                else:
                    nc.vector._custom_dve(
                        ema, out=yt[:, 0:K],
                        in0=xt[:, 0:K].rearrange("p (g n) -> p g n", n=KQ),
                        in1=xt[:, K:2 * K], s0=ca, s1=float(s0v),
                        imm2=imm_a2i)
                if k == n_blocks - 1:
                    # odd half ships at scan end; evens via a short DVE op
                    # (DVE is idle after the last scan; skips PE/PSUM/ACT)
                    outs.append((ybr[r0:r0 + P, K:3 * K], yb[:, K:3 * K]))
                    nc.vector._custom_dve(
                        rec, out=y8[:, 1:K], in0=yt[:, 0:K - 1],
                        in1=xt[:, K + 1:2 * K], s0=float(a0), s1=float(s0v))
                    # fp16 head (col 0 of it is overwritten by y0=x0 on host)
                    nc.vector._custom_dve(
                        rec, out=yh[:, HEAD * k + 1:HEAD * (k + 1)],
                        in0=yt[:, 0:HEAD - 1], in1=xt[:, K + 1:K + HEAD],
                        s0=float(a0), s1=float(s0v))
                    outs.append((ybr[r0:r0 + P, 0:K], yb[:, 0:K]))
                else:
                    recon_evens(k, xt, yt, y8, fine=False)
                    outs.append((ybr[r0:r0 + P, :], yb[:]))
            for dst, src in outs:
                nc.sync.dma_start(dst, src)
            nc.sync.dma_start(h_out.ap(), yh[:])
    nc.compile()
    return nc


def build_fallback(b_per=B_PER, c=C, t=T):
    """Stock-scan kernel (v2): correct for any weights."""
    rows = b_per * c
    n_blocks = rows // P
    c_blocks = c // P
    th = t // 2

    nc = bacc.Bacc("TRN2", target_bir_lowering=False, debug=False)

    x_in = nc.dram_tensor("x", [b_per, c, t], F16, kind="ExternalInput")
    s_in = nc.dram_tensor("s32", [c], F32, kind="ExternalInput")
    a_in = nc.dram_tensor("a16", [c], F16, kind="ExternalInput")
    y_out = nc.dram_tensor("out", [b_per, c, t], F16, kind="ExternalOutput")

    xr = x_in.ap().rearrange("b c t -> (b c) t")
    yr = y_out.ap().rearrange("b c t -> (b c) t")
    sr = s_in.ap().rearrange("(j p) -> p j", p=P)
    ar = a_in.ap().rearrange("(j p) -> p j", p=P)

    with tile.TileContext(nc) as tc:
        with (
            tc.tile_pool(name="const", bufs=1) as cpool,
            tc.tile_pool(name="xp", bufs=6) as xpool,
            tc.tile_pool(name="xh", bufs=4) as hpool,
        ):
            s4 = cpool.tile([P, c_blocks], F32)
            a4 = cpool.tile([P, c_blocks], F16)
            nc.sync.dma_start(s4[:], sr)
            nc.sync.dma_start(a4[:], ar)

            def premul_scan(xt, lo, hi, j, first, init):
                a, b = lo + (1 if first else 0), hi
                nc.scalar.activation(
                    xt[:, a:b], xt[:, a:b], ACT_COPY, scale=s4[:, j:j + 1])
                nc.vector.tensor_tensor_scan(
                    xt[:, lo:hi],
                    a4[:, j:j + 1].to_broadcast((P, hi - lo)),
                    xt[:, lo:hi],
                    init,
                    OP.mult,
                    OP.add,
                )

            split_blocks = (0, n_blocks - 1)
            outs = []
            for k in range(n_blocks):
                j = k % c_blocks
                r0 = k * P
                if k in split_blocks:
                    xa = hpool.tile([P, th], F16)
                    xb = hpool.tile([P, th], F16)
                    nc.sync.dma_start(xa[:], xr[r0:r0 + P, 0:th])
                    nc.sync.dma_start(xb[:], xr[r0:r0 + P, th:t])
                    premul_scan(xa, 0, th, j, True, 0.0)
                    outs.append((yr[r0:r0 + P, 0:th], xa[:]))
                    premul_scan(xb, 0, th, j, False, xa[:, th - 1:th])
                    outs.append((yr[r0:r0 + P, th:t], xb[:]))
                else:
                    xt = xpool.tile([P, t], F16)
                    nc.sync.dma_start(xt[:], xr[r0:r0 + P, :])
                    premul_scan(xt, 0, t, j, True, 0.0)
                    outs.append((yr[r0:r0 + P, :], xt[:]))
            for dst, src in outs:
                nc.sync.dma_start(dst, src)
    nc.compile()
    return nc


_NC_CACHE = {}


def _enable_jax_compile_cache():
    try:
        import jax
        jax.config.update("jax_compilation_cache_dir", "/tmp/jax_neff_cache")
        jax.config.update("jax_persistent_cache_min_compile_time_secs", 1.0)
    except Exception:
        pass


def _get_nc(kind, *args):
    key = (kind,) + tuple(args)
    if key not in _NC_CACHE:
        _enable_jax_compile_cache()
        _NC_CACHE[key] = (build_fast_v2(*args) if kind == "fast"
                          else build_fallback())
    return _NC_CACHE[key]


def _fast_path_ok(s):
    if not np.all(s == s[0]):
        return False
    s0 = float(s[0])
    a0 = 1.0 - s0
    if not (0.0 < s0 < 1.0) or a0 <= 0.0:
        return False
    try:
        lo = a0 ** (T // 2)
        hi = (a0 ** -(T // 2)) * s0 * 125
    except OverflowError:
        return False
    return lo > 1e-37 and hi < 3e37


def kernel(x, weights, _run_kwargs=None):
    x32 = np.asarray(x, dtype=np.float32)
    s = np.clip(np.asarray(weights, dtype=np.float64), 0.0, 1.0)

    if _fast_path_ok(s) and np.abs(x32).max() < 15.0:
        s0 = float(s[0])
        a0 = 1.0 - s0
        xr = x32.reshape(B * C, T)
        x8 = xr.astype(ml_dtypes.float8_e3m4).reshape(B * C, K, 2)
        xe8, xo8 = x8[..., 0], x8[..., 1]
        packed = np.ascontiguousarray(np.concatenate([xo8, xe8], axis=1))
        x0 = xr[:, 0].astype(np.float16).reshape(N_CORES, N_BLOCKS, P) \
            .transpose(0, 2, 1)                       # [core][P, n_blocks]
        da = (np.eye(P) * a0).astype(np.float16)
        ds = (np.eye(P) * s0).astype(np.float16)
        cst = [np.ascontiguousarray(np.concatenate([da, ds, x0[i]], axis=1))
               for i in range(N_CORES)]
        nc = _get_nc("fast", s0)
        in_maps = [
            {"xp": packed[i * ROWS:(i + 1) * ROWS], "cst": cst[i]}
            for i in range(N_CORES)
        ]
        res = run_bass_kernel_spmd(
            nc, in_maps, core_ids=list(range(N_CORES)), **(_run_kwargs or {})
        )
        yb = np.concatenate(
            [np.ascontiguousarray(res.results[i]["yb"])
             for i in range(N_CORES)], axis=0)
        ye = yb[:, 0:K].astype(np.float32)
        yo = yb[:, K:3 * K].view(np.float16).astype(np.float32)
        out = np.empty((B * C, T), dtype=np.float32)
        out[:, 0::2] = ye
        out[:, 1::2] = yo
        # fp16 heads: first HEAD even samples per block
        for i in range(N_CORES):
            yh = res.results[i]["yh"].astype(np.float32)  # [P, HEAD*NB]
            for kb in range(N_BLOCKS):
                rws = slice(i * ROWS + kb * P, i * ROWS + (kb + 1) * P)
                out[rws, 0:2 * HEAD:2] = yh[:, HEAD * kb:HEAD * (kb + 1)]
        out[:, 0] = xr[:, 0]          # y_0 = x_0 exactly
        out = out.reshape(B, C, T)
    else:
        x16 = np.ascontiguousarray(x32.astype(np.float16))
        nc = _get_nc("fallback")
        s32 = s.astype(np.float32)
        a16 = (1.0 - s32).astype(np.float16)
        in_maps = [
            {"x": x16[i * B_PER:(i + 1) * B_PER], "s32": s32, "a16": a16}
            for i in range(N_CORES)
        ]
        res = run_bass_kernel_spmd(
            nc, in_maps, core_ids=list(range(N_CORES)), **(_run_kwargs or {})
        )
        out16 = np.concatenate(
            [res.results[i]["out"] for i in range(N_CORES)], axis=0)
        out = out16.astype(np.float32)
    if _run_kwargs:
        kernel.last_results = res
    return out


# revision 8
# speedup vs baseline: 1.0316x; 1.0198x over previous
"""EMA (first-order linear recurrence along T) for x[16, 512, 4096] f32.

v10 "pair-scan, fp8 both ways": ~41.2-43.4 us HW, rel err 6.0e-3 (gate 2e-2).

Sharding: batch B over 8 cores (1024 rows/core, 8 blocks of 128 partitions).
Wire 10.6 MB/core: in = deinterleaved x planes fp8e3; out = ONE fp8-typed
tensor per row [y_even fp8 | y_odd as raw fp16 bytes] (6 KB rows, single
DMA per block; fp16 region written through an AP bitcast view) plus a tiny
fp16 head (first 32 even samples per block, where a^t*x0 exceeds the fp8
budget; odd plane ships full fp16; y0 = x0 patched on host).

- Odds: custom DVE op EMA_PAIRRS_ANT (hand-authored 5-uop chain): Src0 =
  x_odd, Src1 = x_even as separate streams -> one PAIR per cycle at 1x
  mode. Rescale-trick scans (rho *= a^-2, V += rho*w, h *= a^2, y = V*h)
  in the 8-stage pipe; latch-init derives a = 1-s, a^2 into swap flops;
  per-partition f32 carry rides the s0 const slot; two 1-cycle reseed uops
  at each SUB_DIM_DONE (512-pair piece) renormalize V := V*h, h := 1
  in-pipe, so one instruction scans a whole 2048-pair row.
- Evens on the TensorEngine: psum = diag(a) @ y_odd_shifted(fp16 view) +
  diag(s) @ x_even(fp8); the ACT eviction the evens need anyway writes
  PSUM straight to fp8e3 (free conversion). psum pool bufs=4 (all 8 banks)
  so PE runs a full block ahead of the evictions.
- Overlap: full input prefetch (bufs=n_blocks), one 4 KB-row input DMA per
  block, carries in one DVE copy up front, tail block split (scan halves +
  fine evictions + odd-half output first).
- Perf anatomy: 7.2 preamble + ~1.4 trigger->data + ~29 us DMA-saturated
  payload (10.6 MB at 16 queues) + ~2.5 postamble. DVE 21 us / PE 19 /
  ACT 21 hide under the DMA. Rejected: full-fp8 output (odd-plane
  conversion costs more engine time than it saves), GPSIMD offloads
  (8.8 us/block software CAST, no PSUM access), packed 2x DVE modes.
"""

import numpy as np
import ml_dtypes

import concourse.bacc as bacc
import concourse.mybir as mybir
import concourse.tile as tile
from concourse.bass_utils import run_bass_kernel_spmd

B, C, T = 16, 512, 4096
N_CORES = 8
B_PER = B // N_CORES          # 2 batches per core
ROWS = B_PER * C              # 1024 rows per core
P = 128
N_BLOCKS = ROWS // P          # 8 row blocks
K = T // 2                    # 2048 pairs per row
KP = K // 2                   # 1024 pairs per piece (rescale range limit)
KP2 = KP // 2                 # quarter piece (block-0 fast start)
KQ = 512                      # pairs per in-pipe reseed piece
HEAD = 32                     # fp16 head columns per output plane
MM = 512                      # matmul moving chunk (1 PSUM bank of f32)

F32 = mybir.dt.float32
F16 = mybir.dt.float16
F8 = mybir.dt.float8e3
ACT_COPY = mybir.ActivationFunctionType.Copy
OP = mybir.AluOpType

_OPS = {}


def _register_ema_pair_op(name="EMA_PAIRRS_ANT"):
    """Self-reseeding pair-scan EMA op; see dev_test_reseed.py for layout."""
    from concourse.dve_spec import Spec, Src0, Src1, C0, C1, C2
    from concourse.dve_ops import (
        DveOp, OPS, CUSTOM_DVE_SPECS, _SUB_OPCODE_FOR_NAME,
        _CUSTOM_DVE_ROW_BASE, get_dve_sub_opcode, _COMPILE_CACHE,
    )
    from concourse.dve_uop import (
        DveOpSpec, UopConfig, AluOp, AluInp, InpSel,
        OutPath, OutSel, Trigger, DelayInp, ENABLE,
    )
    if name in _OPS:
        return _OPS[name]

    PREV = AluInp.PREV_ALU_OUT
    CURR = AluInp.CURR_ALU_OUT
    SWAP = AluInp.CURR_SWAP_OUT
    D = lambda n: AluInp(int(AluInp.PREV_DELAY_0) + n)

    # lanes: d0=xo d1=xe d2=a^-2(C2) d3=w then m d4=h
    latch = UopConfig()
    latch.enable_input(InpSel.ONE_F32, 1)
    latch.enable_input(InpSel.CONST_1, 2)
    ldp = latch.datapath_config
    for st in range(8):
        ldp[st].pass_through_delay(0, 1)
    ldp[0].enable_alu(AluOp.SUBTRACT, D(0), D(1))
    ldp[1].enable_alu(AluOp.BYPASS, PREV, PREV)
    ldp[1].swap_enable = ENABLE
    ldp[2].enable_alu(AluOp.MULTIPLY, PREV, PREV)
    for st in (3, 4):
        ldp[st].enable_alu(AluOp.BYPASS, PREV, PREV)
    ldp[5].enable_alu(AluOp.BYPASS, PREV, PREV)
    ldp[5].swap_enable = ENABLE
    latch.repeat_count = 1
    latch.trigger = (Trigger.COUNT, Trigger.NONE, Trigger.NONE)
    latch.next_uop = (1, 0, 0)

    seed = UopConfig()
    seed.enable_input(InpSel.CONST_1, 1)   # d0 = s
    seed.enable_input(InpSel.CONST_0, 2)   # d1 = carry
    seed.enable_input(InpSel.ONE_F32, 3)   # d2 = 1
    sdp = seed.datapath_config
    for st in range(8):
        sdp[st].pass_through_delay(0, 1, 2)
    sdp[3].enable_alu(AluOp.BYPASS, D(0), D(0))   # rho := s
    sdp[5].enable_alu(AluOp.BYPASS, D(2), D(2))   # h := 1
    sdp[6].enable_alu(AluOp.BYPASS, D(1), D(1))   # V := carry
    seed.repeat_count = 1
    seed.trigger = (Trigger.COUNT, Trigger.NONE, Trigger.NONE)
    seed.next_uop = (2, 0, 0)

    steady = UopConfig()
    steady.enable_input(InpSel.SRC_0, 1)
    steady.enable_input(InpSel.SRC_1, 2)
    steady.enable_input(InpSel.CONST_2, 3)
    dp = steady.datapath_config
    for st in range(8):
        dp[st].pass_through_delay(0, 1, 2, 3, 4)
    dp[1].enable_alu(AluOp.MULTIPLY, D(1), SWAP)          # t = a*xe
    dp[2].enable_alu(AluOp.ADD, PREV, D(0))               # w = t+xo
    dp[3].enable_alu(AluOp.MULTIPLY, CURR, D(2))          # rho *= a^-2
    dp[3].enable_delay_from_src(DelayInp.PREV_ALU_OUT, 3)  # d3 := w
    dp[4].enable_alu(AluOp.MULTIPLY, PREV, D(3))          # m = rho*w
    dp[5].enable_alu(AluOp.MULTIPLY, CURR, SWAP)          # h *= a^2
    dp[5].enable_delay_from_src(DelayInp.PREV_ALU_OUT, 3)  # d3 := m
    dp[6].enable_alu(AluOp.ADD, CURR, D(3))               # V += m
    dp[6].enable_delay_from_src(DelayInp.PREV_ALU_OUT, 4)  # d4 := h
    dp[7].enable_alu(AluOp.MULTIPLY, PREV, D(4))          # y = V*h
    steady.enable_output(OutSel.ALU_OUT, OutPath.WR0_LO)
    steady.require_inp0 = ENABLE
    steady.require_inp1 = ENABLE
    steady.trigger = (Trigger.SRC_TENSOR_DONE, Trigger.SUB_DIM_DONE,
                      Trigger.NONE)
    steady.next_uop = (0, 3, 0)

    def mk_rs(st5_op, st6_op, last):
        rs = UopConfig()
        rs.enable_input(InpSel.CONST_1, 1)   # d0 = s
        rs.enable_input(InpSel.ONE_F32, 3)   # d2 = 1
        rdp = rs.datapath_config
        for st in range(8):
            rdp[st].pass_through_delay(0, 1, 2)
        rdp[3].enable_alu(AluOp.BYPASS, D(0), D(0))       # rho := s
        st5_op(rdp[5])
        st6_op(rdp[6])
        rs.repeat_count = 1
        rs.trigger = (Trigger.COUNT, Trigger.NONE, Trigger.NONE)
        rs.next_uop = (4 if not last else 2, 0, 0)
        return rs

    rs1 = mk_rs(
        lambda b: b.enable_alu(AluOp.BYPASS, CURR, CURR),       # h pass
        lambda b: b.enable_alu(AluOp.MULTIPLY, CURR, PREV),     # V *= h_bnd
        last=False)
    rs2 = mk_rs(
        lambda b: b.enable_alu(AluOp.BYPASS, D(2), D(2)),       # h := 1
        lambda b: b.enable_alu(AluOp.BYPASS, CURR, CURR),       # V hold
        last=True)

    def ref(in0, in1, s0, s1, imm2):
        s = np.float64(s1)
        a = 1.0 - s
        w = a * in1.astype(np.float64) + in0.astype(np.float64)
        w = w.reshape(w.shape[0], -1)
        kk = np.arange(w.shape[-1]) + 1.0
        rho = s * (1.0 / a) ** (2 * kk)
        V = np.asarray(s0).reshape(-1, 1) + np.cumsum(rho * w, axis=-1)
        return (V * a ** (2 * kk)).astype(np.float32)

    spec = Spec(body=(Src0 + Src1 * C0) * C1 * C2, reference=ref)
    op = DveOp(name, spec, subdim=True, uops_sha={})
    OPS.append(op)
    CUSTOM_DVE_SPECS[name] = spec
    _SUB_OPCODE_FOR_NAME[name] = _CUSTOM_DVE_ROW_BASE + len(OPS) - 1
    _COMPILE_CACHE[(name, "v3")] = DveOpSpec(
        name=name, opcode=get_dve_sub_opcode(name),
        uops=[latch, seed, steady, rs1, rs2], rd1_en=True)
    _OPS[name] = op
    return op


def _register_recon_op(name="EMA_RECON_ANT"):
    """y_even = a*y_c + s*x_e as a 3-stage DVE op (stock lower()); used for
    the tail block so its evens skip the PE->PSUM->ACT chain after the
    last scan (DVE is idle then)."""
    from concourse.dve_spec import Spec, Src0, Src1, C0, C1, lower
    from concourse.dve_ops import (
        DveOp, OPS, CUSTOM_DVE_SPECS, _SUB_OPCODE_FOR_NAME,
        _CUSTOM_DVE_ROW_BASE, get_dve_sub_opcode, _COMPILE_CACHE,
    )
    from concourse.dve_uop import DveOpSpec
    if name in _OPS:
        return _OPS[name]
    spec = Spec(
        body=Src0 * C0 + Src1 * C1,
        reference=lambda in0, in1, s0, s1, imm2: (
            in0.astype(np.float32) * s0 + in1.astype(np.float32) * s1),
    )
    op = DveOp(name, spec, subdim=False, uops_sha={})
    OPS.append(op)
    CUSTOM_DVE_SPECS[name] = spec
    _SUB_OPCODE_FOR_NAME[name] = _CUSTOM_DVE_ROW_BASE + len(OPS) - 1
    _COMPILE_CACHE[(name, "v3")] = DveOpSpec(
        name=name, opcode=get_dve_sub_opcode(name),
        uops=lower(spec, ver="v3"), rd1_en=True)
    _OPS[name] = op
    return op


def build_fast_v2(s0v, b_per=B_PER, c=C):
    """Pair-scan kernel: fp8 in, fp16 odd plane / fp8 even plane out.

    Odds: self-reseeding DVE pair-scan writes fp16 (also the recon input),
    shipped as-is. Evens: PE diag-matmul recon; the ACT eviction the evens
    need anyway writes PSUM straight to fp8e3 (free conversion), plus a
    small fp16 head (first HEAD even samples per block, where a^t*x0 can
    exceed the fp8 error budget). Wire: 4.2 in + 4.2 + 2.1 + 0.13 out
    = 10.6 MB/core.
    """
    ema = _register_ema_pair_op()
    rec = _register_recon_op()
    a0 = 1.0 - s0v
    rows = b_per * c
    n_blocks = rows // P
    imm_a2i = float((1.0 / np.float64(a0)) ** 2)

    nc = bacc.Bacc("TRN2", target_bir_lowering=False, debug=False)

    x_in = nc.dram_tensor("xp", [rows, 2 * K], F8, kind="ExternalInput")
    cst_in = nc.dram_tensor("cst", [P, 2 * P + n_blocks], F16,
                            kind="ExternalInput")
    yb_out = nc.dram_tensor("yb", [rows, 3 * K], F8, kind="ExternalOutput")
    h_out = nc.dram_tensor("yh", [P, HEAD * n_blocks], F16,
                           kind="ExternalOutput")

    xr = x_in.ap()
    ybr = yb_out.ap()

    with tile.TileContext(nc) as tc:
        with (
            tc.tile_pool(name="const", bufs=1) as cpool,
            tc.tile_pool(name="xpool", bufs=n_blocks) as xpool,
            tc.tile_pool(name="ypool", bufs=n_blocks) as ypool,
            tc.tile_pool(name="y8pool", bufs=n_blocks) as y8pool,
            tc.psum_pool(name="ps", bufs=4) as psp,
        ):
            cst = cpool.tile([P, 2 * P + n_blocks], F16)  # [da | ds | x0]
            cr = cpool.tile([P, n_blocks + 1], F32)
            yh = cpool.tile([P, HEAD * n_blocks], F16)
            nc.sync.dma_start(cst[:], cst_in.ap())
            da = cst[:, 0:P]
            ds = cst[:, P:2 * P]
            x0t = cst[:, 2 * P:2 * P + n_blocks]
            # all piece-1 carries in one DVE copy (DVE is idle pre-scan;
            # keeps ACT free for evictions)
            nc.vector.tensor_copy(cr[:, 0:n_blocks], x0t[:])

            outs = []

            def recon_evens(k, xt, yo, y8, fine):
                """psum = diag(a) @ y_shifted + diag(s) @ x_even -> fp8;
                first chunk also drops an fp16 head of the even samples."""
                step = MM if fine else 2 * MM
                for j in range(0, K, step):
                    ps = psp.tile([P, step], F32)
                    for jj in range(j, j + step, MM):
                        o = jj - j
                        if jj == 0:
                            nc.tensor.matmul(
                                ps[:, 1:MM], da, yo[:, 0:MM - 1],
                                start=True, stop=False)
                        else:
                            nc.tensor.matmul(
                                ps[:, o:o + MM], da,
                                yo[:, jj - 1:jj + MM - 1],
                                start=True, stop=False)
                        nc.tensor.matmul(
                            ps[:, o:o + MM], ds, xt[:, K + jj:K + jj + MM],
                            start=False, stop=True)
                    if j == 0:
                        nc.scalar.activation(
                            yh[:, HEAD * k:HEAD * (k + 1)], ps[:, 0:HEAD],
                            ACT_COPY)
                    nc.scalar.activation(y8[:, j:j + step], ps[:], ACT_COPY)

            for k in range(n_blocks):
                r0 = k * P
                xt = xpool.tile([P, 2 * K], F8)
                nc.sync.dma_start(xt[:], xr[r0:r0 + P, :])

                yb = ypool.tile([P, 3 * K], F8)  # [ye fp8 | yo fp16-bytes]
                yt = yb[:, K:3 * K].bitcast(F16)  # y_odd fp16 view [P, K]
                y8 = yb[:, 0:K]                   # y_even fp8
                ca = cr[:, k:k + 1]
                if k == n_blocks - 1:
                    cq = cr[:, n_blocks:n_blocks + 1]
                    nc.vector._custom_dve(
                        ema, out=yt[:, 0:KP],
                        in0=xt[:, 0:KP].rearrange("p (g n) -> p g n", n=KQ),
                        in1=xt[:, K:K + KP], s0=ca, s1=float(s0v),
                        imm2=imm_a2i)
                    nc.vector.tensor_copy(cq, yt[:, KP - 1:KP])
                    nc.vector._custom_dve(
                        ema, out=yt[:, KP:K],
                        in0=xt[:, KP:K].rearrange("p (g n) -> p g n", n=KQ),
                        in1=xt[:, K + KP:2 * K], s0=cq, s1=float(s0v),
                        imm2=imm_a2i)
                else:
                    nc.vector._custom_dve(
                        ema, out=yt[:, 0:K],
                        in0=xt[:, 0:K].rearrange("p (g n) -> p g n", n=KQ),
                        in1=xt[:, K:2 * K], s0=ca, s1=float(s0v),
                        imm2=imm_a2i)
                if k == n_blocks - 1:
                    # odd half ships at scan end; evens via a short DVE op
                    # (DVE is idle after the last scan; skips PE/PSUM/ACT)
                    outs.append((ybr[r0:r0 + P, K:3 * K], yb[:, K:3 * K]))
                    nc.vector._custom_dve(
                        rec, out=y8[:, 1:K], in0=yt[:, 0:K - 1],
                        in1=xt[:, K + 1:2 * K], s0=float(a0), s1=float(s0v))
                    # fp16 head (col 0 of it is overwritten by y0=x0 on host)
                    nc.vector._custom_dve(
                        rec, out=yh[:, HEAD * k + 1:HEAD * (k + 1)],
                        in0=yt[:, 0:HEAD - 1], in1=xt[:, K + 1:K + HEAD],
                        s0=float(a0), s1=float(s0v))
                    outs.append((ybr[r0:r0 + P, 0:K], yb[:, 0:K]))
                else:
                    recon_evens(k, xt, yt, y8, fine=False)
                    outs.append((ybr[r0:r0 + P, :], yb[:]))
            for dst, src in outs:
                nc.sync.dma_start(dst, src)
            nc.sync.dma_start(h_out.ap(), yh[:])
    nc.compile()
    return nc


def build_fallback(b_per=B_PER, c=C, t=T):
    """Stock-scan kernel (v2): correct for any weights."""
    rows = b_per * c
    n_blocks = rows // P
    c_blocks = c // P
    th = t // 2

    nc = bacc.Bacc("TRN2", target_bir_lowering=False, debug=False)

    x_in = nc.dram_tensor("x", [b_per, c, t], F16, kind="ExternalInput")
    s_in = nc.dram_tensor("s32", [c], F32, kind="ExternalInput")
    a_in = nc.dram_tensor("a16", [c], F16, kind="ExternalInput")
    y_out = nc.dram_tensor("out", [b_per, c, t], F16, kind="ExternalOutput")

    xr = x_in.ap().rearrange("b c t -> (b c) t")
    yr = y_out.ap().rearrange("b c t -> (b c) t")
    sr = s_in.ap().rearrange("(j p) -> p j", p=P)
    ar = a_in.ap().rearrange("(j p) -> p j", p=P)

    with tile.TileContext(nc) as tc:
        with (
            tc.tile_pool(name="const", bufs=1) as cpool,
            tc.tile_pool(name="xp", bufs=6) as xpool,
            tc.tile_pool(name="xh", bufs=4) as hpool,
        ):
            s4 = cpool.tile([P, c_blocks], F32)
            a4 = cpool.tile([P, c_blocks], F16)
            nc.sync.dma_start(s4[:], sr)
            nc.sync.dma_start(a4[:], ar)

            def premul_scan(xt, lo, hi, j, first, init):
                a, b = lo + (1 if first else 0), hi
                nc.scalar.activation(
                    xt[:, a:b], xt[:, a:b], ACT_COPY, scale=s4[:, j:j + 1])
                nc.vector.tensor_tensor_scan(
                    xt[:, lo:hi],
                    a4[:, j:j + 1].to_broadcast((P, hi - lo)),
                    xt[:, lo:hi],
                    init,
                    OP.mult,
                    OP.add,
                )

            split_blocks = (0, n_blocks - 1)
            outs = []
            for k in range(n_blocks):
                j = k % c_blocks
                r0 = k * P
                if k in split_blocks:
                    xa = hpool.tile([P, th], F16)
                    xb = hpool.tile([P, th], F16)
                    nc.sync.dma_start(xa[:], xr[r0:r0 + P, 0:th])
                    nc.sync.dma_start(xb[:], xr[r0:r0 + P, th:t])
                    premul_scan(xa, 0, th, j, True, 0.0)
                    outs.append((yr[r0:r0 + P, 0:th], xa[:]))
                    premul_scan(xb, 0, th, j, False, xa[:, th - 1:th])
                    outs.append((yr[r0:r0 + P, th:t], xb[:]))
                else:
                    xt = xpool.tile([P, t], F16)
                    nc.sync.dma_start(xt[:], xr[r0:r0 + P, :])
                    premul_scan(xt, 0, t, j, True, 0.0)
                    outs.append((yr[r0:r0 + P, :], xt[:]))
            for dst, src in outs:
                nc.sync.dma_start(dst, src)
    nc.compile()
    return nc


_NC_CACHE = {}


def _enable_jax_compile_cache():
    try:
        import jax
        jax.config.update("jax_compilation_cache_dir", "/tmp/jax_neff_cache")
        jax.config.update("jax_persistent_cache_min_compile_time_secs", 1.0)
    except Exception:
        pass


def _get_nc(kind, *args):
    key = (kind,) + tuple(args)
    if key not in _NC_CACHE:
        _enable_jax_compile_cache()
        _NC_CACHE[key] = (build_fast_v2(*args) if kind == "fast"
                          else build_fallback())
    return _NC_CACHE[key]


def _fast_path_ok(s):
    if not np.all(s == s[0]):
        return False
    s0 = float(s[0])
    a0 = 1.0 - s0
    if not (0.0 < s0 < 1.0) or a0 <= 0.0:
        return False
    try:
        lo = a0 ** (T // 2)
        hi = (a0 ** -(T // 2)) * s0 * 125
    except OverflowError:
        return False
    return lo > 1e-37 and hi < 3e37


def kernel(x, weights, _run_kwargs=None):
    x32 = np.asarray(x, dtype=np.float32)
    s = np.clip(np.asarray(weights, dtype=np.float64), 0.0, 1.0)

    if _fast_path_ok(s) and np.abs(x32).max() < 15.0:
        s0 = float(s[0])
        a0 = 1.0 - s0
        xr = x32.reshape(B * C, T)
        x8 = xr.astype(ml_dtypes.float8_e3m4).reshape(B * C, K, 2)
        xe8, xo8 = x8[..., 0], x8[..., 1]
        packed = np.ascontiguousarray(np.concatenate([xo8, xe8], axis=1))
        x0 = xr[:, 0].astype(np.float16).reshape(N_CORES, N_BLOCKS, P) \
            .transpose(0, 2, 1)                       # [core][P, n_blocks]
        da = (np.eye(P) * a0).astype(np.float16)
        ds = (np.eye(P) * s0).astype(np.float16)
        cst = [np.ascontiguousarray(np.concatenate([da, ds, x0[i]], axis=1))
               for i in range(N_CORES)]
        nc = _get_nc("fast", s0)
        in_maps = [
            {"xp": packed[i * ROWS:(i + 1) * ROWS], "cst": cst[i]}
            for i in range(N_CORES)
        ]
        res = run_bass_kernel_spmd(
            nc, in_maps, core_ids=list(range(N_CORES)), **(_run_kwargs or {})
        )
        yb = np.concatenate(
            [np.ascontiguousarray(res.results[i]["yb"])
             for i in range(N_CORES)], axis=0)
        ye = yb[:, 0:K].astype(np.float32)
        yo = yb[:, K:3 * K].view(np.float16).astype(np.float32)
        out = np.empty((B * C, T), dtype=np.float32)
        out[:, 0::2] = ye
        out[:, 1::2] = yo
        # fp16 heads: first HEAD even samples per block
        for i in range(N_CORES):
            yh = res.results[i]["yh"].astype(np.float32)  # [P, HEAD*NB]
            for kb in range(N_BLOCKS):
                rws = slice(i * ROWS + kb * P, i * ROWS + (kb + 1) * P)
                out[rws, 0:2 * HEAD:2] = yh[:, HEAD * kb:HEAD * (kb + 1)]
        out[:, 0] = xr[:, 0]          # y_0 = x_0 exactly
        out = out.reshape(B, C, T)
    else:
        x16 = np.ascontiguousarray(x32.astype(np.float16))
        nc = _get_nc("fallback")
        s32 = s.astype(np.float32)
        a16 = (1.0 - s32).astype(np.float16)
        in_maps = [
            {"x": x16[i * B_PER:(i + 1) * B_PER], "s32": s32, "a16": a16}
            for i in range(N_CORES)
        ]
        res = run_bass_kernel_spmd(
            nc, in_maps, core_ids=list(range(N_CORES)), **(_run_kwargs or {})
        )
        out16 = np.concatenate(
            [res.results[i]["out"] for i in range(N_CORES)], axis=0)
        out = out16.astype(np.float32)
    if _run_kwargs:
        kernel.last_results = res
    return out
